# revision 8
# baseline (speedup 1.0000x reference)
"""Conv-QKV self-attention (CSA) Trainium2 Bass kernel.

Reference computation (per batch b):
    k = conv1d(x, K_w, K_b); q = conv1d(x, Q_w, Q_b); v = conv1d(x, V_w, V_b)
    scores = relu(k^T q)                # [L, L], contraction over 64 channels
    out = v @ scores / sqrt(3)          # [64, L], contraction over L

Sharding: 8 cores = 4 batches x 2 column-halves of the score matrix.
Each core computes k, vT for the full L of its batch, q for its m-half,
then flash-style tiles of relu(k^T q) (kept on-chip) consumed by the
second matmul.  The 1/sqrt(3) is folded into the V weights on the host.
"""

import numpy as np

FIN, FOUT, KS = 64, 64, 3
B, L = 4, 4096
HALF = L // 2            # per-core m range
NCORES = 8
MT = 512                 # m tile (PSUM bank free dim, fp32)
LT = 128                 # l tile (PE partition dim)
N_MT = HALF // MT        # 4
N_LT = L // LT           # 32
SQRT_KS = float(np.sqrt(KS))

# matmul input dtype: "f32r" (full-speed, reduced precision) or "f32"
MM_DTYPE = "f32r"

_NC_CACHE = {}


def _build_nc():
    from contextlib import ExitStack

    import concourse.bass as bass
    import concourse.tile as tile
    from concourse import bacc, mybir

    f32 = mybir.dt.float32
    mm_dt = mybir.dt.float32r if MM_DTYPE == "f32r" else mybir.dt.float32
    AF = mybir.ActivationFunctionType

    nc = bacc.Bacc("TRN2", target_bir_lowering=False)

    xd_d = nc.dram_tensor("xd", [128, L + 2], f32, kind="ExternalInput")
    xq_d = nc.dram_tensor("xq", [128, HALF + 2], f32, kind="ExternalInput")
    kw_d = nc.dram_tensor("kw", [KS, 128, FOUT], f32, kind="ExternalInput")
    qw_d = nc.dram_tensor("qw", [KS, 128, FOUT], f32, kind="ExternalInput")
    vw_d = nc.dram_tensor("vw", [KS, 128, FOUT], f32, kind="ExternalInput")
    kb_d = nc.dram_tensor("kb", [FOUT, 1], f32, kind="ExternalInput")
    qb_d = nc.dram_tensor("qb", [FOUT, 1], f32, kind="ExternalInput")
    vb_d = nc.dram_tensor("vb", [1, FOUT], f32, kind="ExternalInput")
    out_d = nc.dram_tensor("out", [FOUT, HALF], f32, kind="ExternalOutput")

    with tile.TileContext(nc) as tc, ExitStack() as ctx:
        consts = ctx.enter_context(tc.tile_pool(name="consts", bufs=1))
        big = ctx.enter_context(tc.tile_pool(name="big", bufs=1))

        xd_f = consts.tile_from(xd_d[:, :])
        xq_f = consts.tile_from(xq_d[:, :])
        kw_f = consts.tile([128, KS, FOUT], f32)
        nc.sync.dma_start(out=kw_f, in_=kw_d[:, :, :].rearrange("t p c -> p t c"))
        qw_f = consts.tile([128, KS, FOUT], f32)
        nc.sync.dma_start(out=qw_f, in_=qw_d[:, :, :].rearrange("t p c -> p t c"))
        vw_f = consts.tile([128, KS, FOUT], f32)
        nc.sync.dma_start(out=vw_f, in_=vw_d[:, :, :].rearrange("t p c -> p t c"))
        kb_sb = consts.tile_from(kb_d[:, :])
        qb_sb = consts.tile_from(qb_d[:, :])
        vb_sb = consts.tile([128, FOUT], f32)
        nc.sync.dma_start(out=vb_sb, in_=vb_d[:, :].to_broadcast([128, FOUT]))

        # round DMA'd matmul operands to the matmul dtype (walrus requires
        # f32r consumers to read f32r-rounded producers)
        xd_sb = consts.tile([128, L + 2], mm_dt)
        nc.vector.tensor_copy(xd_sb, xd_f)
        xq_sb = consts.tile([128, HALF + 2], mm_dt)
        nc.vector.tensor_copy(xq_sb, xq_f)
        kw_sb = consts.tile([128, KS, FOUT], mm_dt)
        nc.scalar.copy(kw_sb, kw_f)
        qw_sb = consts.tile([128, KS, FOUT], mm_dt)
        nc.scalar.copy(qw_sb, qw_f)
        vw_sb = consts.tile([128, KS, FOUT], mm_dt)
        nc.scalar.copy(vw_sb, vw_f)

        k_sb = big.tile([FIN, L], mm_dt)
        q_sb = big.tile([FIN, HALF], mm_dt)
        vt_sb = big.tile([128, N_LT, FOUT], mm_dt)

        # ---- stage A: conv projections -------------------------------
        actx = ctx.enter_context(ExitStack())
        cpool = actx.enter_context(tc.tile_pool(name="cpsum", bufs=2, space="PSUM"))

        for g in range(L // MT):          # k: [64, L] in natural layout
            pk = cpool.tile([FOUT, MT], f32, name="pk", tag="pkq")
            for t in range(KS):
                nc.tensor.matmul(
                    pk,
                    kw_sb[0:FIN, t, :],
                    xd_sb[0:FIN, g * MT + t : g * MT + t + MT],
                    start=(t == 0),
                    stop=(t == KS - 1),
                )
            nc.scalar.activation(
                k_sb[:, g * MT : (g + 1) * MT], pk, AF.Identity, bias=kb_sb
            )

        for g in range(HALF // MT):       # q: [64, HALF]
            pq = cpool.tile([FOUT, MT], f32, name="pq", tag="pkq")
            for t in range(KS):
                nc.tensor.matmul(
                    pq,
                    qw_sb[0:FIN, t, :],
                    xq_sb[0:FIN, g * MT + t : g * MT + t + MT],
                    start=(t == 0),
                    stop=(t == KS - 1),
                )
            nc.scalar.activation(
                q_sb[:, g * MT : (g + 1) * MT], pq, AF.Identity, bias=qb_sb
            )

        for j in range(N_LT):             # vT: [L, 64] in 128-row tiles
            pv = cpool.tile([128, FOUT], f32, name="pv", tag="pv")
            for t in range(KS):
                nc.tensor.matmul(
                    pv,
                    xd_sb[0:FIN, j * LT + t : j * LT + t + LT],
                    vw_sb[0:FIN, t, :],
                    start=(t == 0),
                    stop=(t == KS - 1),
                )
            nc.vector.tensor_add(vt_sb[:, j, :], pv, vb_sb)

        # ---- stage B: flash loop over score tiles --------------------
        actx.close()
        spsum = ctx.enter_context(tc.tile_pool(name="spsum", bufs=2, space="PSUM"))
        spool = ctx.enter_context(tc.tile_pool(name="spool", bufs=3))
        opsum = ctx.enter_context(tc.tile_pool(name="opsum", bufs=2, space="PSUM"))
        opool = ctx.enter_context(tc.tile_pool(name="opool", bufs=2))

        for mt in range(N_MT):
            po = opsum.tile([FOUT, MT], f32, name="po")
            for lj in range(N_LT // 2):
                ps = spsum.tile([128, 2 * MT], f32, name="ps")
                for h in range(2):
                    lt = 2 * lj + h
                    nc.tensor.matmul(
                        ps[:, h * MT : (h + 1) * MT],
                        k_sb[:, lt * LT : (lt + 1) * LT],
                        q_sb[:, mt * MT : (mt + 1) * MT],
                        start=True,
                        stop=True,
                    )
                s_sb = spool.tile([128, 2 * MT], mm_dt, name="s_sb")
                if lj % 2 == 0:
                    nc.vector.tensor_scalar_max(s_sb, ps, 0.0)
                else:
                    nc.scalar.activation(s_sb, ps, AF.Relu)
                for h in range(2):
                    lt = 2 * lj + h
                    nc.tensor.matmul(
                        po,
                        vt_sb[:, lt, :],
                        s_sb[:, h * MT : (h + 1) * MT],
                        start=(lj == 0 and h == 0),
                        stop=(lj == N_LT // 2 - 1 and h == 1),
                    )
            o_sb = opool.tile([FOUT, MT], f32, name="o_sb")
            nc.scalar.copy(o_sb, po)
            nc.sync.dma_start(out_d[:, mt * MT : (mt + 1) * MT], o_sb)

    nc.finalize()
    return nc


def _get_nc():
    if "nc" not in _NC_CACHE:
        _NC_CACHE["nc"] = _build_nc()
    return _NC_CACHE["nc"]


def make_in_maps(x, K_w, K_b, Q_w, Q_b, V_w, V_b):
    """Host-side marshalling: per-core input dicts for the SPMD kernel."""
    x = np.asarray(x, np.float32)
    xpad = np.zeros((B, FIN, L + 2), np.float32)
    xpad[:, :, 1 : L + 1] = x

    def wT(w):  # [co, ci, t] -> [t, ci, co], duplicated to 128 partitions
        a = np.ascontiguousarray(np.transpose(np.asarray(w, np.float32), (2, 1, 0)))
        return np.concatenate([a, a], axis=1)

    kw = wT(K_w)
    qw = wT(Q_w)
    vw = wT(V_w) / SQRT_KS
    kb = np.asarray(K_b, np.float32).reshape(FOUT, 1)
    qb = np.asarray(Q_b, np.float32).reshape(FOUT, 1)
    vb = (np.asarray(V_b, np.float32) / SQRT_KS).reshape(1, FOUT)

    in_maps = []
    for core in range(NCORES):
        b, h = divmod(core, 2)
        m0 = h * HALF
        xd = np.concatenate([xpad[b], xpad[b]], axis=0)
        xqw = np.ascontiguousarray(xpad[b][:, m0 : m0 + HALF + 2])
        xq = np.concatenate([xqw, xqw], axis=0)
        in_maps.append(
            dict(xd=xd, xq=xq, kw=kw, qw=qw, vw=vw, kb=kb, qb=qb, vb=vb)
        )
    return in_maps


def assemble(results):
    out = np.empty((B, FOUT, L), np.float32)
    for core, res in enumerate(results):
        b, h = divmod(core, 2)
        out[b, :, h * HALF : (h + 1) * HALF] = res["out"]
    return out


def kernel(x, K_w, K_b, Q_w, Q_b, V_w, V_b):
    from concourse.bass_utils import run_bass_kernel_spmd

    nc = _get_nc()
    in_maps = make_in_maps(x, K_w, K_b, Q_w, Q_b, V_w, V_b)
    res = run_bass_kernel_spmd(nc, in_maps, core_ids=list(range(NCORES)))
    return assemble(res.results)


# revision 11
# speedup vs baseline: 1.4135x; 1.4135x over previous
"""Conv-QKV self-attention (CSA) Trainium2 Bass kernel.

Reference computation (per batch b):
    k = conv1d(x, K_w, K_b); q = conv1d(x, Q_w, Q_b); v = conv1d(x, V_w, V_b)
    scores = relu(k^T q)                # [L, L], contraction over 64 channels
    out = v @ scores / sqrt(3)          # [64, L], contraction over L

Sharding: 8 cores = 4 batches x 2 row-halves (l) of the score matrix.
Each core computes k, vT for its l-half, q for the full L, a flash-style
pass over relu(k^T q) tiles, and a PARTIAL out (contraction over its
l-half).  The host sums the two partials per batch.  1/sqrt(3) is folded
into the V weights on the host.

PE packing: the score matmul (K=64) runs as row-packed pairs (two
concurrent 64-row-group matmuls on duplicated k/q partitions); the
output matmul (M=64) runs as col-packed pairs (two m-tiles into psum
partitions 0:64 / 64:128).  The PE stream is software-pipelined so the
mm2 pair of iteration i issues after the mm1 pair of iteration i+1,
hiding the relu (PSUM->SBUF) latency.
"""

import numpy as np

FIN, FOUT, KS = 64, 64, 3
B, L = 4, 4096
HALF = L // 2            # per-core l range
NCORES = 8
MT = 512                 # m tile (PSUM bank free dim, fp32)
LT = 128                 # l tile (PE partition dim)
N_MT = L // MT           # 8  (full m range per core)
N_LT = HALF // LT        # 16 (l tiles in this core's half)
SQRT_KS = float(np.sqrt(KS))

# matmul input dtype: "f32r" (2 cyc/row, ~tf32 precision) or "f32"
MM_DTYPE = "f32r"

_NC_CACHE = {}


def _build_nc():
    from contextlib import ExitStack

    import concourse.tile as tile
    from concourse import bacc, mybir

    f32 = mybir.dt.float32
    mm_dt = mybir.dt.float32r if MM_DTYPE == "f32r" else mybir.dt.float32
    AF = mybir.ActivationFunctionType

    nc = bacc.Bacc("TRN2", target_bir_lowering=False)

    # xk: this core's l-window of x (+/-1 halo); xd: full x (+/-1 pad)
    xk_d = nc.dram_tensor("xk", [FIN, HALF + 2], f32, kind="ExternalInput")
    xd_d = nc.dram_tensor("xd", [FIN, L + 2], f32, kind="ExternalInput")
    kw_d = nc.dram_tensor("kw", [KS, FIN, FOUT], f32, kind="ExternalInput")
    qw_d = nc.dram_tensor("qw", [KS, FIN, FOUT], f32, kind="ExternalInput")
    vw_d = nc.dram_tensor("vw", [KS, FIN, FOUT], f32, kind="ExternalInput")
    kb_d = nc.dram_tensor("kb", [FOUT, 1], f32, kind="ExternalInput")
    qb_d = nc.dram_tensor("qb", [FOUT, 1], f32, kind="ExternalInput")
    vb_d = nc.dram_tensor("vb", [1, FOUT], f32, kind="ExternalInput")
    out_d = nc.dram_tensor("out", [FOUT, L], f32, kind="ExternalOutput")

    with tile.TileContext(nc) as tc, ExitStack() as ctx:
        consts = ctx.enter_context(tc.tile_pool(name="consts", bufs=1))
        big = ctx.enter_context(tc.tile_pool(name="big", bufs=1))

        xk_f = consts.tile([FIN, HALF + 2], f32)
        nc.sync.dma_start(out=xk_f, in_=xk_d[:, :])
        xd_f = consts.tile([FIN, L + 2], f32)
        nc.gpsimd.dma_start(out=xd_f, in_=xd_d[:, :])
        kw_f = consts.tile([FIN, KS, FOUT], f32)
        nc.sync.dma_start(out=kw_f, in_=kw_d[:, :, :].rearrange("t p c -> p t c"))
        qw_f = consts.tile([FIN, KS, FOUT], f32)
        nc.sync.dma_start(out=qw_f, in_=qw_d[:, :, :].rearrange("t p c -> p t c"))
        vw_f = consts.tile([FIN, KS, FOUT], f32)
        nc.sync.dma_start(out=vw_f, in_=vw_d[:, :, :].rearrange("t p c -> p t c"))
        kb_sb = consts.tile([FOUT, 1], f32)
        nc.sync.dma_start(out=kb_sb, in_=kb_d[:, :])
        qb_sb = consts.tile([FOUT, 1], f32)
        nc.sync.dma_start(out=qb_sb, in_=qb_d[:, :])
        vb_sb = consts.tile([128, FOUT], f32)
        nc.sync.dma_start(out=vb_sb, in_=vb_d[:, :].to_broadcast([128, FOUT]))

        # round DMA'd matmul operands to the matmul dtype (walrus requires
        # f32r consumers to read f32r-rounded producers)
        xk_sb = consts.tile([FIN, HALF + 2], mm_dt)
        nc.vector.tensor_copy(xk_sb, xk_f)
        xd_sb = consts.tile([FIN, L + 2], mm_dt)
        nc.vector.tensor_copy(xd_sb, xd_f)
        kw_sb = consts.tile([FIN, KS, FOUT], mm_dt)
        nc.scalar.copy(kw_sb, kw_f)
        qw_sb = consts.tile([FIN, KS, FOUT], mm_dt)
        nc.scalar.copy(qw_sb, qw_f)
        vw_sb = consts.tile([FIN, KS, FOUT], mm_dt)
        nc.scalar.copy(vw_sb, vw_f)

        # k (this half) and q (full), duplicated across both partition
        # halves for the row-packed score matmuls
        k2_sb = big.tile([128, HALF], mm_dt)
        q2_sb = big.tile([128, L], mm_dt)
        vt_sb = big.tile([128, N_LT, FOUT], mm_dt)

        # ---- stage A: conv projections -------------------------------
        actx = ctx.enter_context(ExitStack())
        cpool = actx.enter_context(tc.tile_pool(name="cpsum", bufs=2, space="PSUM"))

        for g in range(HALF // MT):       # k: [64, HALF]
            pk = cpool.tile([FOUT, MT], f32, name="pk", tag="pkq")
            for t in range(KS):
                nc.tensor.matmul(
                    pk,
                    kw_sb[:, t, :],
                    xk_sb[:, g * MT + t : g * MT + t + MT],
                    start=(t == 0),
                    stop=(t == KS - 1),
                )
            nc.scalar.activation(
                k2_sb[0:FOUT, g * MT : (g + 1) * MT], pk, AF.Identity, bias=kb_sb
            )

        for g in range(L // MT):          # q: [64, L]
            pq = cpool.tile([FOUT, MT], f32, name="pq", tag="pkq")
            for t in range(KS):
                nc.tensor.matmul(
                    pq,
                    qw_sb[:, t, :],
                    xd_sb[:, g * MT + t : g * MT + t + MT],
                    start=(t == 0),
                    stop=(t == KS - 1),
                )
            nc.scalar.activation(
                q2_sb[0:FOUT, g * MT : (g + 1) * MT], pq, AF.Identity, bias=qb_sb
            )

        for j in range(N_LT):             # vT: [HALF, 64] in 128-row tiles
            pv = cpool.tile([128, FOUT], f32, name="pv", tag="pv")
            for t in range(KS):
                nc.tensor.matmul(
                    pv,
                    xk_sb[:, j * LT + t : j * LT + t + LT],
                    vw_sb[:, t, :],
                    start=(t == 0),
                    stop=(t == KS - 1),
                )
            nc.vector.tensor_add(vt_sb[:, j, :], pv, vb_sb)

        # duplicate k and q into partitions 64:128 (DMA, off engines)
        nc.sync.dma_start(out=k2_sb[FOUT:128, :], in_=k2_sb[0:FOUT, :])
        nc.gpsimd.dma_start(out=q2_sb[FOUT:128, :], in_=q2_sb[0:FOUT, :])

        # ---- stage B: flash loop over score tiles --------------------
        actx.close()
        spsum = ctx.enter_context(tc.tile_pool(name="spsum", bufs=2, space="PSUM"))
        spool = ctx.enter_context(tc.tile_pool(name="spool", bufs=3))
        opsum = ctx.enter_context(tc.tile_pool(name="opsum", bufs=2, space="PSUM"))
        opool = ctx.enter_context(tc.tile_pool(name="opool", bufs=2))

        for mp in range(N_MT // 2):
            mtA, mtB = 2 * mp, 2 * mp + 1
            poA = opsum.tile([FOUT, MT], f32, name="poA", tag="poA")
            poB = opsum.tile([FOUT, MT], f32, name="poB", tag="poB")
            pending = None  # (lj, s_sb) awaiting its mm2 pair
            for lj in range(N_LT):
                ps = spsum.tile([128, 2 * MT], f32, name="ps")
                # mm1 pair: row-packed (K=64 each) on duplicated k/q
                nc.tensor.matmul(
                    ps[:, 0:MT],
                    k2_sb[0:FOUT, lj * LT : (lj + 1) * LT],
                    q2_sb[0:FOUT, mtA * MT : (mtA + 1) * MT],
                    start=True,
                    stop=True,
                    tile_position=(0, 0),
                )
                nc.tensor.matmul(
                    ps[:, MT : 2 * MT],
                    k2_sb[FOUT:128, lj * LT : (lj + 1) * LT],
                    q2_sb[FOUT:128, mtB * MT : (mtB + 1) * MT],
                    start=True,
                    stop=True,
                    tile_position=(64, 0),
                )
                # software pipeline: issue previous iteration's mm2 pair
                # now, so the PE isn't blocked on this iteration's relu
                if pending is not None:
                    plj, ps_sb = pending
                    nc.tensor.matmul(
                        poA,
                        vt_sb[:, plj, :],
                        ps_sb[:, 0:MT],
                        start=(plj == 0),
                        stop=False,
                    )
                    nc.tensor.matmul(
                        poB,
                        vt_sb[:, plj, :],
                        ps_sb[:, MT : 2 * MT],
                        start=(plj == 0),
                        stop=False,
                    )
                s_sb = spool.tile([128, 2 * MT], mm_dt, name="s_sb")
                if lj % 2 == 0:
                    nc.vector.tensor_scalar_max(s_sb, ps, 0.0)
                else:
                    nc.scalar.activation(s_sb, ps, AF.Relu)
                pending = (lj, s_sb)

            plj, ps_sb = pending
            nc.tensor.matmul(
                poA, vt_sb[:, plj, :], ps_sb[:, 0:MT], start=(plj == 0), stop=True
            )
            nc.tensor.matmul(
                poB,
                vt_sb[:, plj, :],
                ps_sb[:, MT : 2 * MT],
                start=(plj == 0),
                stop=True,
            )
            o_sbA = opool.tile([FOUT, MT], f32, name="o_sbA", tag="oA")
            nc.scalar.copy(o_sbA, poA)
            nc.sync.dma_start(out_d[:, mtA * MT : (mtA + 1) * MT], o_sbA)
            o_sbB = opool.tile([FOUT, MT], f32, name="o_sbB", tag="oB")
            nc.scalar.copy(o_sbB, poB)
            nc.sync.dma_start(out_d[:, mtB * MT : (mtB + 1) * MT], o_sbB)

    nc.finalize()
    return nc


def _get_nc():
    if "nc" not in _NC_CACHE:
        _NC_CACHE["nc"] = _build_nc()
    return _NC_CACHE["nc"]


def make_in_maps(x, K_w, K_b, Q_w, Q_b, V_w, V_b):
    """Host-side marshalling: per-core input dicts for the SPMD kernel."""
    x = np.asarray(x, np.float32)
    xpad = np.zeros((B, FIN, L + 2), np.float32)
    xpad[:, :, 1 : L + 1] = x

    def wT(w):  # [co, ci, t] -> [t, ci, co]
        return np.ascontiguousarray(
            np.transpose(np.asarray(w, np.float32), (2, 1, 0))
        )

    kw = wT(K_w)
    qw = wT(Q_w)
    vw = wT(V_w) / SQRT_KS
    kb = np.asarray(K_b, np.float32).reshape(FOUT, 1)
    qb = np.asarray(Q_b, np.float32).reshape(FOUT, 1)
    vb = (np.asarray(V_b, np.float32) / SQRT_KS).reshape(1, FOUT)

    in_maps = []
    for core in range(NCORES):
        b, h = divmod(core, 2)
        l0 = h * HALF
        xk = np.ascontiguousarray(xpad[b][:, l0 : l0 + HALF + 2])
        in_maps.append(
            dict(xk=xk, xd=xpad[b], kw=kw, qw=qw, vw=vw, kb=kb, qb=qb, vb=vb)
        )
    return in_maps


def assemble(results):
    out = np.empty((B, FOUT, L), np.float32)
    for b in range(B):
        out[b] = results[2 * b]["out"] + results[2 * b + 1]["out"]
    return out


def kernel(x, K_w, K_b, Q_w, Q_b, V_w, V_b):
    from concourse.bass_utils import run_bass_kernel_spmd

    nc = _get_nc()
    in_maps = make_in_maps(x, K_w, K_b, Q_w, Q_b, V_w, V_b)
    res = run_bass_kernel_spmd(nc, in_maps, core_ids=list(range(NCORES)))
    return assemble(res.results)
